# revision 1
# baseline (speedup 1.0000x reference)
"""BinaryLinear Trainium2 kernel: y = x @ sign(W).T + bias.

Contract: kernel(x, weight, bias) takes FULL unsharded numpy inputs
(x [32768,1024] f32, weight [1024,1024] f32, bias [1024] f32) and returns
the FULL output [32768,1024] f32.

Strategy (8 NeuronCores, data-parallel over tokens):
  - x is sharded into 8 x [4096, 1024] row shards; weight+bias replicated.
  - Per core, everything happens on-device:
      * weight prep: DMA W, PE-transpose 128x128 blocks, ACT Sign -> bf16
        wT [i, o] tiles. sign(W) in {-1,0,+1} is exactly representable in
        bf16, so the binarized matmul loses nothing from bf16 weights.
      * x is streamed in fp32, PE-transposed (i onto partitions), then split
        into x_hi = bf16(xT) (ACT cast) and x_lo = bf16(xT - x_hi) (DVE sub).
        x == x_hi + x_lo to ~bf16(lo) precision (rel ~2^-17), so
        y = x_hi @ s + x_lo @ s accumulated in fp32 PSUM is ~fp32-accurate
        while running the PE at full bf16 rate (fp32 matmul is 4x slower).
      * 16 bf16 matmuls (8 K-chunks x {hi,lo}) accumulate each [128, 512]
        PSUM tile; DVE adds the (DMA-broadcast) bias on eviction.
  - Emission is software-pipelined one macro-tile deep so PE never waits on
    the DVE splits of the tile it is about to multiply.
"""

import numpy as np

import concourse.bass as bass  # noqa: F401  (bass types used via bacc)
import concourse.mybir as mybir
import concourse.tile as tile
from concourse import bacc
from concourse.bass_utils import run_bass_kernel_spmd
from concourse.masks import make_identity

P = 128
N_CORES = 8
F32 = mybir.dt.float32
BF16 = mybir.dt.bfloat16


def build_kernel(
    ntok: int,
    d: int,
    o: int,
    macro: int = 512,
    two_pass: bool = True,
    lo_fp8: bool = True,
):
    """Build the per-core Bass program for x [ntok, d] f32 -> y [ntok, o] f32."""
    assert ntok % macro == 0 and macro % P == 0 and d % P == 0 and o % P == 0
    NS = macro // P  # token subtiles per macro tile
    NM = ntok // macro  # macro tiles
    IC = d // P  # contraction chunks
    OC = o // P  # output-feature 128-blocks (weight prep granularity)
    OGW = min(512, o)  # matmul free dim / psum bank width
    NOG = o // OGW
    WG = min(4, OC)  # weight-prep transpose blocks per psum tile
    FP8 = mybir.dt.float8e5  # lo-pass dtype: e5m2 (no denormal trouble for |lo|<=2^-9|x|)
    lo_fp8 = lo_fp8 and two_pass and IC % 2 == 0

    nc = bacc.Bacc(None, target_bir_lowering=False)

    x = nc.dram_tensor("x", [ntok, d], F32, kind="ExternalInput")
    w = nc.dram_tensor("w", [o, d], F32, kind="ExternalInput")
    bias = nc.dram_tensor("bias", [1, o], F32, kind="ExternalInput")
    y = nc.dram_tensor("y", [ntok, o], F32, kind="ExternalOutput")

    xr = x[:].rearrange("(m s p) d -> p m s d", p=P, s=NS)
    yr = y[:].rearrange("(m s p) o -> p m s o", p=P, s=NS)
    wr = w[:].rearrange("(oc p) d -> p oc d", p=P)

    with tile.TileContext(nc) as tc:
        with (
            tc.tile_pool(name="const", bufs=1) as const,
            tc.tile_pool(name="wstage", bufs=1) as wstage,
            tc.tile_pool(name="xpool", bufs=2) as xpool,
            tc.tile_pool(name="xtpool", bufs=2) as xtpool,
            tc.tile_pool(name="ypool", bufs=4) as ypool,
            tc.tile_pool(name="tpsum", bufs=2, space="PSUM") as tpsum,
            tc.tile_pool(name="ypsum", bufs=4, space="PSUM") as ypsum,
        ):
            # ---- constants ----
            ident = const.tile([P, P], F32)
            make_identity(nc, ident)
            bias_bc = const.tile([P, o], F32)
            nc.scalar.dma_start(bias_bc[:], bias[:].to_broadcast((P, o)))

            # ---- weight prep: wT[i, o] = sign(W[o, i]), bf16 (hi) + fp8 (lo) ----
            # Two contiguous row-chunks; prep per chunk so PE work overlaps the
            # second chunk's DMA. Emitted after split_section(0) (see below).
            wT = const.tile([P, IC, o], BF16)
            wT8 = const.tile([P, IC, o], FP8, name="wT8") if lo_fp8 else None
            w_sb = wstage.tile([P, OC, d], F32)

            def weight_dma(g):
                ocs = slice(g, g + WG)
                nc.sync.dma_start(w_sb[:, ocs], wr[:, ocs])

            def weight_prep(g):
                for ic in range(IC):
                    isl = slice(ic * P, (ic + 1) * P)
                    pw = tpsum.tile([P, WG * P], F32, tag="pw")
                    for j in range(WG):
                        nc.tensor.transpose(
                            pw[:, j * P : (j + 1) * P],
                            w_sb[:, g + j, isl],
                            ident[:],
                        )
                    osl = slice(g * P, (g + WG) * P)
                    nc.scalar.sign(wT[:, ic, osl], pw[:])
                    if lo_fp8:
                        nc.scalar.sign(wT8[:, ic, osl], pw[:])

            # HAM warm-up: ~4us of dummy matmuls during the startup DMA wait
            # so the first (DMA-gated) transposes run at 2.4 GHz, not 1.2.
            dummy = const.tile([P, 512], BF16, name="dummy")
            nc.gpsimd.memset(dummy[:], 0.0)
            dpsum = ypsum.tile([P, OGW], F32, tag="yp", name="ypdummy")
            for _ in range(10):
                nc.tensor.matmul(
                    dpsum[:], dummy[:, :P], dummy[:, :OGW], start=True, stop=True
                )

            # ---- main loop, software-pipelined one macro deep ----
            lo_dt = FP8 if lo_fp8 else BF16
            prev = None  # (hiT, loT) awaiting their matmul section

            def split_section(m):
                x_sb = xpool.tile([P, NS, d], F32, tag="x_sb")
                # x0 on the SWDGE queue (parallel with the weight DMA on sync);
                # later tiles go on sync BEHIND the weights so the x prefetch
                # can't starve the weight load the first matmuls wait on.
                dma = nc.gpsimd if m == 0 else nc.sync
                dma.dma_start(x_sb[:, :, : d // 2], xr[:, m, :, : d // 2])
                dma.dma_start(x_sb[:, :, d // 2 :], xr[:, m, :, d // 2 :])
                hiT = xtpool.tile([P, IC, macro], BF16, tag="hiT")
                loT = xtpool.tile([P, IC, macro], lo_dt, tag="loT")
                for ic in range(IC):
                    pt = tpsum.tile([P, macro], F32, tag="pt")
                    for s in range(NS):
                        nc.tensor.transpose(
                            pt[:, s * P : (s + 1) * P],
                            x_sb[:, s, ic * P : (ic + 1) * P],
                            ident[:],
                        )
                    # hi = bf16(xT) on ACT; lo = fp8/bf16(xT - hi) on DVE
                    nc.scalar.copy(hiT[:, ic], pt[:])
                    if two_pass:
                        nc.vector.tensor_tensor(
                            loT[:, ic], pt[:], hiT[:, ic], mybir.AluOpType.subtract
                        )
                return hiT, loT

            def mm_section(m, hiT, loT):
                for s in range(NS):
                    tok = slice(s * P, (s + 1) * P)
                    y_sb = ypool.tile([P, o], F32, tag="y_sb")
                    yps = [
                        ypsum.tile([P, OGW], F32, tag="yp", name=f"yp{og}")
                        for og in range(NOG)
                    ]
                    if two_pass and lo_fp8:
                        # all hi (bf16) matmuls for both output groups, then
                        # all lo DoubleRow matmuls as one burst: the DR
                        # LDWEIGHTS pipeline-fill (~400ns) is paid once per
                        # burst, not once per group.
                        for og in range(NOG):
                            osl = slice(og * OGW, (og + 1) * OGW)
                            for ic in range(IC):
                                nc.tensor.matmul(
                                    yps[og][:],
                                    hiT[:, ic, tok],
                                    wT[:, ic, osl],
                                    start=(ic == 0),
                                    stop=False,
                                )
                        for og in range(NOG):
                            osl = slice(og * OGW, (og + 1) * OGW)
                            for ic in range(0, IC, 2):
                                nc.tensor.matmul(
                                    yps[og][:],
                                    loT[:, ic : ic + 2, tok],
                                    wT8[:, ic : ic + 2, osl],
                                    start=False,
                                    stop=(ic == IC - 2),
                                    perf_mode=mybir.MatmulPerfMode.DoubleRow,
                                )
                    else:
                        for og in range(NOG):
                            osl = slice(og * OGW, (og + 1) * OGW)
                            for ic in range(IC):
                                nc.tensor.matmul(
                                    yps[og][:],
                                    hiT[:, ic, tok],
                                    wT[:, ic, osl],
                                    start=(ic == 0),
                                    stop=(not two_pass and ic == IC - 1),
                                )
                            if two_pass:
                                for ic in range(IC):
                                    nc.tensor.matmul(
                                        yps[og][:],
                                        loT[:, ic, tok],
                                        wT[:, ic, osl],
                                        start=False,
                                        stop=(ic == IC - 1),
                                    )
                    for og in range(NOG):
                        osl = slice(og * OGW, (og + 1) * OGW)
                        nc.vector.tensor_tensor(
                            y_sb[:, osl], yps[og][:], bias_bc[:, osl], mybir.AluOpType.add
                        )
                    nc.scalar.dma_start(yr[:, m, s], y_sb[:])

            def mm_first(hiT, loT):
                # macro 0, og-major: og=0 matmuls need only the first weight
                # chunk's prep; the second chunk's prep slots between the og
                # passes (its DMA long done), off the startup critical path.
                ysb = {
                    s: ypool.tile([P, o], F32, tag="y_sb", name=f"ysbf{s}")
                    for s in range(NS)
                }
                for og in range(NOG):
                    if og >= 1 and og * WG < OC:
                        weight_prep(og * WG)
                    osl = slice(og * OGW, (og + 1) * OGW)
                    for s in range(NS):
                        tok = slice(s * P, (s + 1) * P)
                        yp = ypsum.tile([P, OGW], F32, tag="yp", name=f"ypf{s % 2}")
                        for ic in range(IC):
                            nc.tensor.matmul(
                                yp[:],
                                hiT[:, ic, tok],
                                wT[:, ic, osl],
                                start=(ic == 0),
                                stop=(not two_pass and ic == IC - 1),
                            )
                        if two_pass and lo_fp8:
                            for ic in range(0, IC, 2):
                                nc.tensor.matmul(
                                    yp[:],
                                    loT[:, ic : ic + 2, tok],
                                    wT8[:, ic : ic + 2, osl],
                                    start=False,
                                    stop=(ic == IC - 2),
                                    perf_mode=mybir.MatmulPerfMode.DoubleRow,
                                )
                        elif two_pass:
                            for ic in range(IC):
                                nc.tensor.matmul(
                                    yp[:],
                                    loT[:, ic, tok],
                                    wT[:, ic, osl],
                                    start=False,
                                    stop=(ic == IC - 1),
                                )
                        nc.vector.tensor_tensor(
                            ysb[s][:, osl], yp[:], bias_bc[:, osl], mybir.AluOpType.add
                        )
                # any weight chunks not covered by an og pass (small configs)
                for g in range(max(1, NOG) * WG, OC, WG):
                    weight_prep(g)
                for s in range(NS):
                    nc.scalar.dma_start(yr[:, 0, s], ysb[s][:])

            for m in range(NM + 1):
                if m == 1:
                    # emit macro 0's matmuls BEFORE split(1): otherwise T(1)
                    # (gated on the x1 DMA, queued behind the 4 MiB weight
                    # load) sits ahead of MM(0) in the PE FIFO and blocks it
                    mm_first(*prev)
                if m < NM:
                    cur = split_section(m)
                if m == 0:
                    # weight DMAs issued up front (ahead of x1+ on the sync
                    # ring); first chunk prepped now, second inside mm_first
                    for g in range(0, OC, WG):
                        weight_dma(g)
                    weight_prep(0)
                if m >= 2:
                    mm_section(m - 1, *prev)
                if m < NM:
                    prev = cur

    nc.compile()
    return nc


_NC_CACHE: dict = {}


def _get_nc(ntok, d, o):
    key = (ntok, d, o)
    if key not in _NC_CACHE:
        _NC_CACHE[key] = build_kernel(ntok, d, o)
    return _NC_CACHE[key]


def kernel(x, weight, bias):
    x = np.ascontiguousarray(np.asarray(x, dtype=np.float32))
    weight = np.ascontiguousarray(np.asarray(weight, dtype=np.float32))
    bias = np.ascontiguousarray(np.asarray(bias, dtype=np.float32))
    ntok, d = x.shape
    o = weight.shape[0]
    assert ntok % N_CORES == 0
    shard = ntok // N_CORES

    nc = _get_nc(shard, d, o)
    bias2d = bias.reshape(1, o)
    in_maps = [
        {"x": x[i * shard : (i + 1) * shard], "w": weight, "bias": bias2d}
        for i in range(N_CORES)
    ]
    res = run_bass_kernel_spmd(nc, in_maps, core_ids=list(range(N_CORES)))
    return np.concatenate([r["y"] for r in res.results], axis=0)



# revision 15
# speedup vs baseline: 1.5140x; 1.5140x over previous
"""BinaryLinear Trainium2 kernel: y = x @ sign(W).T + bias.

Contract: kernel(x, weight, bias) takes FULL unsharded numpy inputs
(x [32768,1024] f32, weight [1024,1024] f32, bias [1024] f32) and returns
the FULL output [32768,1024] f32.

Strategy (8 NeuronCores, data-parallel over tokens):
  - x is sharded into 8 x [4096, 1024] row shards; weight+bias replicated.
  - Per core, everything happens on-device:
      * weight prep: DMA W (1 MiB chunks), PE-transpose 128x128 blocks,
        ACT Sign -> bf16 wT [i, o]. sign(W) in {-1,0,+1} is exact in bf16.
      * x pipeline per macro tile: DMA f32 -> DVE cast to bf16 ->
        PE-transpose in bf16 (1 cyc/row vs 2 for f32) -> ACT evicts PSUM
        to SBUF xT tiles -> single bf16 matmul pass accumulated in f32
        PSUM. bf16 x contributes ~1e-3 rel error vs the 2e-2 tolerance;
        a second (lo) pass would double PE time for nothing.
      * DVE adds bias on PSUM eviction; y is written bf16 (~1e-3 rel) and
        upcast to f32 on the host in the gather step, halving out traffic.
  - Macro schedule [1,2,4,...,4,1] token-tiles: small first macros shorten
    the startup dependency chain (DMA->cast->transpose->evict->matmul),
    a small last macro shortens the drain tail. Steady state is PE-bound
    with matmuls back-to-back (HAM stays warm).
"""

import numpy as np

import concourse.bass as bass  # noqa: F401  (bass types used via bacc)
import concourse.mybir as mybir
import concourse.tile as tile
from concourse import bacc
from concourse.bass_utils import run_bass_kernel_spmd
from concourse.masks import make_identity

P = 128
N_CORES = 8
F32 = mybir.dt.float32
BF16 = mybir.dt.bfloat16

OUT_BF16 = True
XPOSE_BF16 = True


def _schedule(T, big=4, ramp=(1, 1, 2), tail=(1,)):
    """Macro sizes in token-tiles: ramp up, steady, short tail."""
    sched = list(ramp)
    rem = T - sum(ramp) - sum(tail)
    while rem >= big:
        sched.append(big)
        rem -= big
    if rem:
        sched.append(rem)
    sched.extend(tail)
    assert sum(sched) == T
    return sched


def build_kernel(
    ntok: int,
    d: int,
    o: int,
    out_bf16: bool = OUT_BF16,
    xpose_bf16: bool = XPOSE_BF16,
    bufs: int = 3,
):
    """Build the per-core Bass program for x [ntok, d] f32 -> y [ntok, o]."""
    assert ntok % P == 0 and d % P == 0 and o % P == 0
    T = ntok // P  # token 128-tiles
    IC = d // P  # contraction chunks
    OC = o // P  # output-feature 128-blocks
    OGW = min(512, o)  # matmul free dim / psum bank width
    NOG = o // OGW
    WG = min(2, OC)  # weight-prep blocks per psum tile / DMA chunk
    NSMAX = 4
    sched = _schedule(T, NSMAX)
    XDT = BF16 if xpose_bf16 else F32
    YDT = BF16 if out_bf16 else F32

    nc = bacc.Bacc(None, target_bir_lowering=False)

    x = nc.dram_tensor("x", [ntok, d], F32, kind="ExternalInput")
    w = nc.dram_tensor("w", [o, d], F32, kind="ExternalInput")
    bias = nc.dram_tensor("bias", [1, o], F32, kind="ExternalInput")
    y = nc.dram_tensor("y", [ntok, o], YDT, kind="ExternalOutput")

    xr = x[:].rearrange("(t p) d -> p t d", p=P)
    yr = y[:].rearrange("(t p) o -> p t o", p=P)
    wr = w[:].rearrange("(oc p) d -> p oc d", p=P)

    with tile.TileContext(nc) as tc:
        with (
            tc.tile_pool(name="const", bufs=1) as const,
            tc.tile_pool(name="wstage", bufs=1) as wstage,
            tc.tile_pool(name="xbpool", bufs=bufs) as xbpool,
            tc.tile_pool(name="xtpool", bufs=bufs) as xtpool,
            tc.tile_pool(name="ypool", bufs=4) as ypool,
            tc.tile_pool(name="tpsum", bufs=2, space="PSUM") as tpsum,
            tc.tile_pool(name="ypsum", bufs=4, space="PSUM") as ypsum,
        ):
            # ---- constants ----
            ident = const.tile([P, P], F32)
            make_identity(nc, ident)
            if xpose_bf16:
                ident_x = const.tile([P, P], BF16, name="identb")
                nc.scalar.copy(ident_x[:], ident[:])
            else:
                ident_x = ident
            bias_bc = const.tile([P, o], F32)
            nc.scalar.dma_start(bias_bc[:], bias[:].to_broadcast((P, o)))

            # ---- weight prep: wT[i, o] = sign(W[o, i]), bf16 ----
            wT = const.tile([P, IC, o], BF16)
            w_sb = wstage.tile([P, OC, d], F32)

            def weight_dma(g):
                # chunks alternate between the two HWDGE rings so the ones the
                # first matmuls need don't queue behind the whole 4 MiB load
                ocs = slice(g, g + WG)
                eng = nc.sync if (g // WG) % 2 == 0 else nc.scalar
                eng.dma_start(w_sb[:, ocs], wr[:, ocs])

            def weight_prep(g):
                for ic in range(IC):
                    isl = slice(ic * P, (ic + 1) * P)
                    pw = tpsum.tile([P, WG * P], F32, tag="pw")
                    for j in range(WG):
                        nc.tensor.transpose(
                            pw[:, j * P : (j + 1) * P],
                            w_sb[:, g + j, isl],
                            ident[:],
                        )
                    osl = slice(g * P, (g + WG) * P)
                    nc.scalar.sign(wT[:, ic, osl], pw[:])

            # HAM warm-up: ~4us of dummy matmuls during the startup DMA wait
            # so the first (DMA-gated) transposes run at 2.4 GHz, not 1.2.
            dummy = const.tile([P, 512], BF16, name="dummy")
            nc.gpsimd.memset(dummy[:], 0.0)
            dpsum = ypsum.tile([P, OGW], F32, tag="yp", name="ypdummy")
            for _ in range(10):
                nc.tensor.matmul(
                    dpsum[:], dummy[:, :P], dummy[:, :OGW], start=True, stop=True
                )

            # ---- main loop, software-pipelined one macro deep ----
            def split_section(m, t0, ns):
                # SWDGE casts f32 -> bf16 inline during the DMA: no on-chip
                # cast pass, half the SBUF write traffic, shorter dep chain
                src = xbpool.tile([P, NSMAX, d], XDT, tag="xb")
                if ns == 1:
                    nc.gpsimd.dma_start(src[:, :1], xr[:, t0 : t0 + 1])
                else:
                    nc.gpsimd.dma_start(
                        src[:, :ns, : d // 2], xr[:, t0 : t0 + ns, : d // 2]
                    )
                    nc.gpsimd.dma_start(
                        src[:, :ns, d // 2 :], xr[:, t0 : t0 + ns, d // 2 :]
                    )
                xt = xtpool.tile([P, IC, NSMAX * P], BF16, tag="xT")
                for ic in range(IC):
                    pt = tpsum.tile([P, NSMAX * P], XDT, tag="pt")
                    for s in range(ns):
                        nc.tensor.transpose(
                            pt[:, s * P : (s + 1) * P],
                            src[:, s, ic * P : (ic + 1) * P],
                            ident_x[:],
                        )
                    nc.scalar.copy(xt[:, ic, : ns * P], pt[:, : ns * P])
                return xt

            def mm_chain(yp, xt, tok, osl):
                for ic in range(IC):
                    nc.tensor.matmul(
                        yp[:],
                        xt[:, ic, tok],
                        wT[:, ic, osl],
                        start=(ic == 0),
                        stop=(ic == IC - 1),
                    )

            def mm_section(m, t0, ns, xt):
                # one batched y DMA per macro: the per-transfer ~2us fixed
                # cost would otherwise eat the scalar ring (and the tail)
                y_sb = ypool.tile([P, NSMAX, o], YDT, tag="y_sb")
                for s in range(ns):
                    tok = slice(s * P, (s + 1) * P)
                    yps = [
                        ypsum.tile([P, OGW], F32, tag="yp", name=f"yp{og}")
                        for og in range(NOG)
                    ]
                    for og in range(NOG):
                        mm_chain(yps[og], xt, tok, slice(og * OGW, (og + 1) * OGW))
                    for og in range(NOG):
                        osl = slice(og * OGW, (og + 1) * OGW)
                        nc.vector.tensor_tensor(
                            y_sb[:, s, osl], yps[og][:], bias_bc[:, osl],
                            mybir.AluOpType.add,
                        )
                nc.scalar.dma_start(yr[:, t0 : t0 + ns], y_sb[:, :ns])

            def mm_first(t0, ns, xt):
                # macro 0 (one token tile) runs in WG*P-wide output chunks,
                # each gated only on its own weight chunk, so matmuls start
                # as soon as the first weight DMA lands instead of after
                # half the load.
                assert ns == 1
                tok = slice(0, P)
                y_sb = ypool.tile([P, NSMAX, o], YDT, tag="y_sb", name="ysbf")
                for c in range(o // (WG * P)):
                    if c > 0:
                        weight_prep(c * WG)
                    osl = slice(c * WG * P, (c + 1) * WG * P)
                    yp = ypsum.tile([P, WG * P], F32, tag="yp", name=f"ypf{c % 2}")
                    mm_chain(yp[:], xt, tok, osl)
                    nc.vector.tensor_tensor(
                        y_sb[:, 0, osl], yp[:], bias_bc[:, osl],
                        mybir.AluOpType.add,
                    )
                nc.scalar.dma_start(yr[:, t0 : t0 + ns], y_sb[:, :ns])

            NM = len(sched)
            t0s = np.cumsum([0] + sched).tolist()
            prev = None
            for m in range(NM + 1):
                if m == 1:
                    # emit macro 0's matmuls BEFORE split(1) so the (DMA-
                    # gated) transposes of macro 1 don't block them in the
                    # PE FIFO
                    mm_first(t0s[0], sched[0], prev)
                if m < NM:
                    cur = split_section(m, t0s[m], sched[m])
                if m == 0:
                    # weight DMAs issued up front (split across both HWDGE
                    # rings); the first chunk preps now, the rest pipeline
                    # inside mm_first as their DMAs land
                    for g in range(0, OC, WG):
                        weight_dma(g)
                    weight_prep(0)
                if m >= 2:
                    mm_section(m - 1, t0s[m - 1], sched[m - 1], prev)
                if m < NM:
                    prev = cur

    nc.compile()
    return nc


_NC_CACHE: dict = {}


def _get_nc(ntok, d, o):
    key = (ntok, d, o)
    if key not in _NC_CACHE:
        _NC_CACHE[key] = build_kernel(ntok, d, o)
    return _NC_CACHE[key]


def kernel(x, weight, bias):
    x = np.ascontiguousarray(np.asarray(x, dtype=np.float32))
    weight = np.ascontiguousarray(np.asarray(weight, dtype=np.float32))
    bias = np.ascontiguousarray(np.asarray(bias, dtype=np.float32))
    ntok, d = x.shape
    o = weight.shape[0]
    assert ntok % N_CORES == 0
    shard = ntok // N_CORES

    nc = _get_nc(shard, d, o)
    bias2d = bias.reshape(1, o)
    in_maps = [
        {"x": x[i * shard : (i + 1) * shard], "w": weight, "bias": bias2d}
        for i in range(N_CORES)
    ]
    res = run_bass_kernel_spmd(nc, in_maps, core_ids=list(range(N_CORES)))
    return np.concatenate(
        [np.asarray(r["y"], dtype=np.float32) for r in res.results], axis=0
    )
